# revision 27
# baseline (speedup 1.0000x reference)
"""Trainium2 Bass kernel for nn_Enet_81037442941606 (gnn_message_passing).

Computation (reference):
    g   = enc_out[batch_idx, tgt]                      # [N, D] gather
    h0  = batchnorm(g)  (training stats, biased var)   # [N, D]
    h1  = swish(h0 @ wt2_w.T + wt2_b)                  # [N, C]
    out = h1 @ A.T + h1   (A sparse, NNZ entries)      # [N, C]

Strategy (8 NeuronCores, tensor parallel over the class axis):
  * Each core owns a contiguous block of C/8 = 8192 classes: its wt2_w rows,
    its A rows (spmm output rows), and its output columns.
  * Host pre-transposes the W shard so the device reads perfect [d, c] tiles,
    and packs the sparse matrix as per-row-block selection matrices + column
    gather indices (pure data-layout transforms of A).
  * Device: token gather, PE-transpose of the activations, batchnorm stats
    along the free axis, in-place normalize; bf16 main matmul producing the
    h1^T shard (bf16 resident in SBUF); two chunked bf16 AllGathers
    overlapping the matmul; then the spmm as batched dma_gather ops (512+
    rows per op, int16 indices, spread over 4 SWDGE queues) feeding
    selection-matrix matmuls accumulating in PSUM, fused residual add,
    bf16 transposed output shard out.
  * Host concatenates the 8 output shards and transposes back to [N, C].
"""

import numpy as np
import ml_dtypes

import concourse.bacc as bacc
import concourse.bass as bass
import concourse.mybir as mybir
import concourse.tile as tile
from concourse.bass_utils import run_bass_kernel_spmd
from concourse.masks import make_identity

# Problem sizes (hardcoded per contest rules).
B, S, D, C, N = 32, 128, 1024, 65536, 512
NNZ = 262144
EPS = 1e-5
NCORES = 8
CLOC = C // NCORES          # classes per core = 8192
NB = CLOC // 128            # 64 row-blocks per core
ND = D // 128               # 8 contraction chunks
NT = N // 128               # 4 token tiles
P = 128

EX_DT = mybir.dt.bfloat16   # h1 exchange dtype
EX_NP = ml_dtypes.bfloat16
MM_DT = mybir.dt.bfloat16   # main-matmul operand dtype (W, h0^T)
MM_NP = ml_dtypes.bfloat16

# AllGather runs as 2 chunked collectives ([24, 40] local c-tiles): per-CC
# fixed overhead (~40-80us barrier+SPAD) makes finer chunking slower, and
# this split lets the first CC overlap the matmul tail. ag_out row space is
# chunk-major (chunk 0 rows [0, 24576) rank-major, chunk 1 rows
# [24576, 65536) rank-major). The int16 gather-half boundary at row 32768
# falls inside chunk 1; the class permutation is constrained to keep each
# class's half fixed (region-preserving packing, see _prep_host).
AGT = (24, 40)              # tiles per AG chunk
AGR = tuple(t * P for t in AGT)      # rows per rank per chunk (3072, 5120)
AGB = (0, NCORES * AGR[0])  # global row base of each chunk
HALF = C // 2               # 32768-row halves for int16 dma_gather indices
# blocks of each rank whose classes are half-0 sources (see ag row formula):
# rank 0: all 64; rank 1: blocks < 48; ranks 2-7: blocks < 24.
KAPPA = (64, 48, 24, 24, 24, 24, 24, 24)

def ag_row_like(rr, lnew):
    """Global ag_out row of owner rank rr, permuted local index lnew."""
    lnew = np.asarray(lnew)
    return np.where(lnew < AGR[0], rr * AGR[0] + lnew,
                    AGB[1] + rr * AGR[1] + (lnew - AGR[0]))


_PROGRAM_CACHE = {}
TRACE = False          # set by test.py to capture an NTFF profile
LAST_RESULTS = None    # BassKernelResults of the last kernel() call


def _build_program(profile: tuple):
    """Build + compile the SPMD Bass program (identical on all 8 cores).

    profile[rb] = (c0, c1): number of 128-row gather chunks sourced from
    ag_out half 0 / half 1 for row block rb (same on every core; per-core
    data is padded to it).
    """
    if profile in _PROGRAM_CACHE:
        return _PROGRAM_CACHE[profile]
    tot_ch = sum(c0 + c1 for c0, c1 in profile)
    idx_cols = tot_ch * (P // 16)
    # block pairs: (chunk columns, idx cols) laid out pair-major as
    # [h0(b0) h0(b1) h1(b0) h1(b1)] so each half needs ONE dma_gather per
    # pair (fewer, bigger gather ops).
    NPAIR = NB // 2          # int16 idx cols overall

    nc = bacc.Bacc("TRN2", target_bir_lowering=False, debug=False,
                   num_devices=NCORES, num_swdge_queues=4,
                   dynamic_dma_scratch_size=24576)
    f32 = mybir.dt.float32
    i32 = mybir.dt.int32
    i16 = mybir.dt.int16

    enc = nc.dram_tensor("enc", [B * S, D], f32, kind="ExternalInput")
    gidx = nc.dram_tensor("gidx", [P, NT], i32, kind="ExternalInput")
    wt = nc.dram_tensor("wt", [NB, P, D], MM_DT, kind="ExternalInput")
    biasv = nc.dram_tensor("biasv", [P, NB], f32, kind="ExternalInput")
    sel = nc.dram_tensor("sel", [P, tot_ch * P], EX_DT, kind="ExternalInput")
    gidx16 = nc.dram_tensor("gidx16", [P, idx_cols], i16, kind="ExternalInput")
    outT = nc.dram_tensor("outT", [CLOC, N], EX_DT, kind="ExternalOutput")

    ag_ins = [nc.dram_tensor(f"ag_in{k}", [AGR[k], N], EX_DT)
              for k in range(2)]
    ag_out = nc.dram_tensor("ag_out", [C, N], EX_DT, addr_space="Shared")
    ag_out_ch = [ag_out[AGB[0]:AGB[1], :], ag_out[AGB[1]:C, :]]
    ag_half = [ag_out[0:HALF, :], ag_out[HALF:C, :]]
    ag_in_vs = [t.ap().rearrange("(i p) n -> i p n", p=P) for t in ag_ins]
    outT_v = outT.ap().rearrange("(i p) n -> i p n", p=P)

    with tile.TileContext(nc) as tc:
        with (
            tc.tile_pool(name="persist", bufs=1) as persist,
        ):
            h1T = persist.tile([P, NB * N], EX_DT)      # [c%128, (ctile, n)]
            bias_t = persist.tile([P, NB], f32)
            gidx16_t = persist.tile([P, idx_cols], i16)
            ident = persist.tile([P, P], f32)
            mean_s = persist.tile([P, ND], f32)
            rstd_s = persist.tile([P, ND], f32)

            make_identity(nc, ident[:])
            nc.sync.dma_start(out=bias_t[:], in_=biasv[:])
            nc.sync.dma_start(out=gidx16_t[:], in_=gidx16[:])

            gidx_t = persist.tile([P, NT], i32)
            nc.sync.dma_start(out=gidx_t[:], in_=gidx[:])

            # W tile loads batched 4 c-tiles per DMA (1 MB) to keep PE fed;
            # the pool opens before phase A so the first loads prefetch early.
            WB = 4
            wt_b = wt.ap().rearrange("(a b) p d -> a b p d", b=WB)
            with (
                tc.tile_pool(name="phW", bufs=4) as phW,
                tc.tile_pool(name="phA", bufs=1) as phA,
                tc.tile_pool(name="psA", bufs=4, space="PSUM") as psA,
            ):
                h0T = phA.tile([P, ND * N], MM_DT)      # [d%128, (dchunk, n)]
                wt_tiles = []
                for a in range(3):      # prefetch first groups during A
                    wt_a = phW.tile([P, WB * D], MM_DT, tag="wt")
                    nc.sync.dma_start(
                        out=wt_a[:].rearrange("p (b d) -> p b d", b=WB),
                        in_=wt_b[a].rearrange("b p d -> p b d"))
                    wt_tiles.append(wt_a)

                # ---------------- Phase A: gather + batchnorm + h0^T -------
                g_tiles = []
                for j in range(NT):
                    g_j = phA.tile([P, D], f32, tag=f"g{j}")
                    nc.gpsimd.indirect_dma_start(
                        out=g_j[:], out_offset=None, in_=enc[:],
                        in_offset=bass.IndirectOffsetOnAxis(
                            ap=gidx_t[:, j:j + 1], axis=0),
                    )
                    g_tiles.append(g_j)

                # Raw transpose g -> h0T (tokens on the free axis)
                for j in range(NT):
                    for i in range(ND):
                        tp = psA.tile([P, P], f32, space="PSUM", tag="tp")
                        nc.tensor.transpose(
                            tp[:], g_tiles[j][:, i * P:(i + 1) * P], ident[:])
                        nc.vector.tensor_copy(
                            out=h0T[:, i * N + j * P: i * N + (j + 1) * P],
                            in_=tp[:])

                # Batch stats along the free (token) axis via ACT accum_out
                sum_s = phA.tile([P, ND], f32, tag="sums")
                sq_s = phA.tile([P, ND], f32, tag="sqs")
                scr = phA.tile([P, N], f32, tag="scr")
                for i in range(ND):
                    nc.scalar.activation(
                        scr[:], h0T[:, i * N:(i + 1) * N],
                        mybir.ActivationFunctionType.Copy,
                        accum_out=sum_s[:, i:i + 1])
                    nc.scalar.activation(
                        scr[:], h0T[:, i * N:(i + 1) * N],
                        mybir.ActivationFunctionType.Square,
                        accum_out=sq_s[:, i:i + 1])

                ex2_s = phA.tile([P, ND], f32, tag="ex2")
                var_s = phA.tile([P, ND], f32, tag="var")
                nc.scalar.mul(mean_s[:], sum_s[:], 1.0 / N)
                nc.scalar.mul(ex2_s[:], sq_s[:], 1.0 / N)
                nc.vector.tensor_tensor(
                    out=var_s[:], in0=mean_s[:], in1=mean_s[:],
                    op=mybir.AluOpType.mult)
                nc.vector.tensor_tensor(
                    out=var_s[:], in0=ex2_s[:], in1=var_s[:],
                    op=mybir.AluOpType.subtract)
                sd_s = phA.tile([P, ND], f32, tag="sd")
                epsb = phA.tile([P, 1], f32, tag="epsb")
                nc.vector.memset(epsb[:], EPS)
                nc.scalar.activation(
                    sd_s[:], var_s[:], mybir.ActivationFunctionType.Sqrt,
                    bias=epsb[:, :1], scale=1.0)
                nc.vector.reciprocal(rstd_s[:], sd_s[:])

                for i in range(ND):
                    nc.vector.tensor_scalar(
                        out=h0T[:, i * N:(i + 1) * N],
                        in0=h0T[:, i * N:(i + 1) * N],
                        scalar1=mean_s[:, i:i + 1],
                        scalar2=rstd_s[:, i:i + 1],
                        op0=mybir.AluOpType.subtract,
                        op1=mybir.AluOpType.mult,
                    )

                # ---------------- Phase B: h1^T = swish(W h0^T + b) --------
                with (
                    tc.tile_pool(name="psB", bufs=4, space="PSUM") as psB,
                ):
                    for a in range(NB // WB):
                        if a < 3:
                            wt_a = wt_tiles[a]
                        else:
                            wt_a = phW.tile([P, WB * D], MM_DT, tag="wt")
                            nc.sync.dma_start(
                                out=wt_a[:].rearrange("p (b d) -> p b d", b=WB),
                                in_=wt_b[a].rearrange("b p d -> p b d"))
                        for bsub in range(WB):
                            i = a * WB + bsub
                            h1ps = psB.tile([P, N], f32, space="PSUM",
                                            tag="h1ps")
                            for k in range(ND):
                                nc.tensor.matmul(
                                    out=h1ps[:],
                                    lhsT=wt_a[:, bsub * D + k * P:
                                              bsub * D + (k + 1) * P],
                                    rhs=h0T[:, k * N:(k + 1) * N],
                                    start=(k == 0), stop=(k == ND - 1),
                                )
                            nc.scalar.activation(
                                h1T[:, i * N:(i + 1) * N], h1ps[:],
                                mybir.ActivationFunctionType.Silu,
                                bias=bias_t[:, i:i + 1], scale=1.0)
                            k_ag = 0 if i < AGT[0] else 1
                            nc.sync.dma_start(
                                out=ag_in_vs[k_ag][i - (AGT[0] if k_ag else 0)],
                                in_=h1T[:, i * N:(i + 1) * N])

                        # ---- Phase C: chunked AllGathers fire as soon as
                        # their class sub-range of h1^T has been written out.
                        if (a + 1) * WB == AGT[0]:
                            k_ag = 0
                        elif (a + 1) * WB == NB:
                            k_ag = 1
                        else:
                            k_ag = None
                        if k_ag is not None:
                            nc.gpsimd.collective_compute(
                                "AllGather",
                                mybir.AluOpType.bypass,
                                replica_groups=[list(range(NCORES))],
                                ins=[ag_ins[k_ag][:].opt()],
                                outs=[ag_out_ch[k_ag].opt()],
                            )

            # ---------------- Phase D: spmm + residual ---------------------
            with (
                tc.tile_pool(name="phD", bufs=6) as phD,
                tc.tile_pool(name="ctp", bufs=9) as ctp,
                tc.tile_pool(name="psD", bufs=8, space="PSUM") as psD,
            ):
                cpmax = max(sum(profile[2 * p][i] + profile[2 * p + 1][i]
                                for i in range(2)) for p in range(NPAIR))
                ch_off = 0      # chunk offset (for sel + idx addressing)
                qn = 0
                for pr in range(NPAIR):
                    b0, b1 = 2 * pr, 2 * pr + 1
                    c0a, c1a = profile[b0]
                    c0b, c1b = profile[b1]
                    ch0 = c0a + c0b          # half-0 chunks of the pair
                    ch1 = c1a + c1b
                    cp = ch0 + ch1
                    ct = ctp.tile([P, cpmax * N], EX_DT, tag="ct")
                    for h, (coff, chn) in enumerate(((0, ch0), (ch0, ch1))):
                        icol0 = (ch_off + coff) * (P // 16)
                        nc.gpsimd.dma_gather(
                            ct[:, coff * N:(coff + chn) * N].rearrange(
                                "p (c e) -> p c e", c=chn),
                            ag_half[h],
                            gidx16_t[:, icol0:icol0 + chn * (P // 16)],
                            chn * P,
                            chn * P,
                            N,
                            queue_num=qn % 4,
                        )
                        qn += 1
                    sel_t = phD.tile([P, cpmax * P], EX_DT, tag="sel")
                    nc.sync.dma_start(
                        out=sel_t[:, :cp * P],
                        in_=sel[:, ch_off * P:(ch_off + cp) * P])
                    # block b0 chunks: [0, c0a) and [ch0, ch0+c1a);
                    # block b1 chunks: [c0a, ch0) and [ch0+c1a, cp).
                    # Issue all half-0 matmuls (both blocks) first so the PE
                    # can start as soon as the h0 gather lands, then half-1.
                    accs = [psD.tile([P, N], f32, space="PSUM", tag="acc",
                                     name=f"acc{pr}_{i}")
                            for i in range(2)]
                    chsets = [
                        (list(range(0, c0a)), list(range(ch0, ch0 + c1a))),
                        (list(range(c0a, ch0)), list(range(ch0 + c1a, cp))),
                    ]
                    for half in range(2):
                        for bi in range(2):
                            chs_h = chsets[bi][half]
                            first = half == 0
                            last_ch = chsets[bi][1][-1]
                            for j, ch in enumerate(chs_h):
                                nc.tensor.matmul(
                                    out=accs[bi][:],
                                    lhsT=sel_t[:, ch * P:(ch + 1) * P],
                                    rhs=ct[:, ch * N:(ch + 1) * N],
                                    start=(first and j == 0),
                                    stop=(ch == last_ch),
                                )
                    for bi, rb in enumerate((b0, b1)):
                        o_t = phD.tile([P, N], EX_DT, tag="ot")
                        nc.vector.tensor_tensor(
                            out=o_t[:], in0=accs[bi][:],
                            in1=h1T[:, rb * N:(rb + 1) * N],
                            op=mybir.AluOpType.add)
                        nc.scalar.dma_start(out=outT_v[rb], in_=o_t[:])
                    ch_off += cp

    nc.compile()
    _PROGRAM_CACHE[profile] = nc
    return nc


def _pack_blocks(deg0, deg1, profile):
    """Greedy 2D bin packing of CLOC rows into NB blocks.

    deg0/deg1: per-row source counts in ag_out half 0 / half 1.
    profile[b] = (c0, c1) chunk capacities (x128 slots each).
    Returns (assign, slot) row->block, row->slot-in-block or None.
    """
    nb = len(profile)
    nrows = len(deg0)
    cap0 = np.array([c0 * P for c0, _ in profile], dtype=np.int64)
    cap1 = np.array([c1 * P for _, c1 in profile], dtype=np.int64)
    order = np.argsort(-(deg0 + deg1), kind="stable")
    loads0 = np.zeros(nb, dtype=np.int64)
    loads1 = np.zeros(nb, dtype=np.int64)
    cnts = np.zeros(nb, dtype=np.int64)
    assign = np.empty(nrows, dtype=np.int64)
    slot = np.empty(nrows, dtype=np.int64)
    for row in order:
        d0, d1 = deg0[row], deg1[row]
        s0 = (loads0 + d0) / cap0
        s1 = (loads1 + d1) / cap1
        score = np.maximum(s0, s1)
        bad = (cnts >= P) | (loads0 + d0 > cap0) | (loads1 + d1 > cap1)
        score[bad] = np.inf
        b = int(np.argmin(score))
        if not np.isfinite(score[b]):
            return None, None
        assign[row] = b
        slot[row] = cnts[b]
        loads0[b] += d0
        loads1[b] += d1
        cnts[b] += 1
    return assign, slot


def _prep_host(enc_out, wt2_w, wt2_b, A_values, batch_idx, tgt, A_indices):
    """Shard inputs + restructure the sparse matrix for the device program."""
    enc_flat = np.ascontiguousarray(
        np.asarray(enc_out, dtype=np.float32).reshape(B * S, D))
    flat_idx = (np.asarray(batch_idx, dtype=np.int64) * S
                + np.asarray(tgt, dtype=np.int64)).astype(np.int32)
    gidx_host = np.ascontiguousarray(flat_idx.reshape(NT, P).T)

    wt2_w = np.asarray(wt2_w, dtype=np.float32)
    wt2_b = np.asarray(wt2_b, dtype=np.float32)
    rows_all = np.asarray(A_indices[0], dtype=np.int64)
    cols_all = np.asarray(A_indices[1], dtype=np.int64)
    vals_all = np.asarray(A_values, dtype=np.float32)

    # ag_out global row for a source class of owner rank rr at permuted
    # local index lnew (chunk-major [24, 40] layout):
    #   lnew <  AGR[0]: row = rr*AGR[0] + lnew
    #   lnew >= AGR[0]: row = AGB[1] + rr*AGR[1] + (lnew - AGR[0])
    # Half 0 (row < HALF) iff lnew < KAPPA[rr]*128, with KAPPA above.
    def ag_row_of(rr, lnew):
        k0 = lnew < AGR[0]
        return np.where(k0, rr * AGR[0] + lnew,
                        AGB[1] + rr * AGR[1] + (lnew - AGR[0]))

    # Per-rank sparse slices.
    rank_data = []
    for r in range(NCORES):
        m = (rows_all // CLOC) == r
        rl = (rows_all[m] - r * CLOC).astype(np.int64)
        cc = cols_all[m]
        vv = vals_all[m]
        rank_data.append((rl, cc, vv))

    # The gather half of a source class depends on its PERMUTED local index
    # (half 0 iff lnew < KAPPA[rr]*128), but the permutation comes from the
    # block packing, which itself needs per-dst-row half-degrees.  Break the
    # cycle by making the permutation REGION-PRESERVING: tentatively assign
    # class c of rank rr to half H(c) := [l_identity < KAPPA[rr]*128], then
    # pack each rank's region-A classes only into blocks [0, KAPPA[r]) and
    # region-B classes into [KAPPA[r], NB) (counts match exactly), so the
    # tentative halves are exact.
    nfat = 16
    while True:
        # fat (3,3) blocks spread evenly so every region prefix has slack.
        prof = [(2, 2)] * NB
        for i in range(nfat):
            prof[(i * NB + NB // 2) // nfat] = (3, 3)
        profile = tuple(prof)
        perms = []
        ok = True
        for r in range(NCORES):
            rl, cc, vv = rank_data[r]
            src_rr = cc // CLOC
            src_l = cc % CLOC
            kap = np.asarray(KAPPA, dtype=np.int64)[src_rr] * P
            src_half = (src_l >= kap).astype(np.int64)
            deg0 = np.bincount(rl[src_half == 0], minlength=CLOC)
            deg1 = np.bincount(rl[src_half == 1], minlength=CLOC)
            # pack each region independently into its block range
            assign = np.empty(CLOC, dtype=np.int64)
            slot = np.empty(CLOC, dtype=np.int64)
            kr = KAPPA[r]
            for h, (lo, hi, blo, bhi) in enumerate(
                    ((0, kr * P, 0, kr), (kr * P, CLOC, kr, NB))):
                if lo == hi:
                    continue
                subprof = profile[blo:bhi]
                a, s = _pack_blocks(deg0[lo:hi], deg1[lo:hi], subprof)
                if a is None:
                    ok = False
                    break
                assign[lo:hi] = a + blo
                slot[lo:hi] = s
            if not ok:
                break
            perms.append(assign * P + slot)     # old local -> new local
        if ok:
            break
        nfat += 8
        if nfat > NB:
            raise RuntimeError("packing failed")

    chunks = [c0 + c1 for c0, c1 in profile]
    tot_ch = sum(chunks)
    idx_cols = tot_ch * (P // 16)
    # pair-major chunk offsets: pair pr holds, in order,
    # [h0(b0): c0a][h0(b1): c0b][h1(b0): c1a][h1(b1): c1b]
    NPAIR = NB // 2
    pair_cp = [sum(profile[2 * p]) + sum(profile[2 * p + 1])
               for p in range(NPAIR)]
    pair_off = np.zeros(NPAIR, dtype=np.int64)
    pair_off[1:] = np.cumsum(pair_cp)[:-1]
    # chunk-column base of (block, half) within the global layout
    blkh_base = np.zeros((NB, 2), dtype=np.int64)
    for p_ in range(NPAIR):
        b0_, b1_ = 2 * p_, 2 * p_ + 1
        c0a_, c1a_ = profile[b0_]
        c0b_, c1b_ = profile[b1_]
        blkh_base[b0_, 0] = pair_off[p_]
        blkh_base[b1_, 0] = pair_off[p_] + c0a_
        blkh_base[b0_, 1] = pair_off[p_] + c0a_ + c0b_
        blkh_base[b1_, 1] = pair_off[p_] + c0a_ + c0b_ + c1a_

    new2old = [np.argsort(p) for p in perms]

    per_rank = []
    for r in range(NCORES):
        rl, cc, vv = rank_data[r]
        rl_new = perms[r][rl]

        # global ag_out row of each contribution's source
        rr = cc // CLOC
        lnew_src = np.empty(len(cc), dtype=np.int64)
        for r2 in range(NCORES):
            m2 = rr == r2
            lnew_src[m2] = perms[r2][cc[m2] % CLOC]
        ag_rows = ag_row_of(rr, lnew_src)
        src_h = (ag_rows >= HALF).astype(np.int64)
        # order contributions by (block, half, ag_row)
        blk = rl_new // P
        order = np.lexsort((ag_rows, src_h, blk))
        blk = blk[order]
        src_h_o = src_h[order]
        ag_rows_o = ag_rows[order]
        vv_o = vv[order]
        dst_o = (rl_new % P)[order]

        sel_host = np.zeros((P, tot_ch * P), dtype=EX_NP)
        gidx16_host = np.zeros((P, idx_cols), dtype=np.int16)

        for b in range(NB):
            c0, c1 = profile[b]
            mb = blk == b
            for h, chn in enumerate((c0, c1)):
                mh = mb & (src_h_o == h)
                nrows = int(mh.sum())
                assert nrows <= chn * P, (b, h, nrows, chn * P)
                idx_local = (ag_rows_o[mh] - h * HALF).astype(np.int16)
                padded = np.zeros(chn * P, dtype=np.int16)
                padded[:nrows] = idx_local
                # wrapped int16 layout: idx i at [i%16, i//16], replicated
                wrapped = padded.reshape(chn * (P // 16), 16).T
                base = blkh_base[b, h]
                icol0 = base * (P // 16)
                gidx16_host[:, icol0:icol0 + chn * (P // 16)] = np.tile(
                    wrapped, (NCORES, 1))
                # sel layout: partition = slot-within-chunk, column =
                # chunk_col*P + dst_row (lhsT of the selection matmul)
                slots = np.arange(nrows)
                sel_host[slots % P,
                         (base + slots // P) * P + dst_o[mh]] = \
                    vv_o[mh].astype(EX_NP)

        rows = slice(r * CLOC, (r + 1) * CLOC)
        wr = wt2_w[rows][new2old[r]]  # [8192, 1024] in permuted order
        wt_host = np.ascontiguousarray(
            wr.reshape(NB, P, ND, P).transpose(0, 3, 2, 1)
        ).reshape(NB, P, D).astype(MM_NP)
        bias_host = np.ascontiguousarray(
            wt2_b[rows][new2old[r]].reshape(NB, P).T)
        per_rank.append({
            "enc": enc_flat,
            "gidx": gidx_host,
            "wt": wt_host,
            "biasv": bias_host,
            "sel": sel_host,
            "gidx16": gidx16_host,
        })
    return per_rank, profile, new2old


def kernel(**inputs) -> np.ndarray:
    per_rank, profile, new2old = _prep_host(
        inputs["enc_out"], inputs["wt2_w"], inputs["wt2_b"],
        inputs["A_values"], inputs["batch_idx"], inputs["tgt"],
        inputs["A_indices"])
    nc = _build_program(profile)
    res = None
    last_exc = None
    for _attempt in range(3):
        try:
            res = run_bass_kernel_spmd(
                nc, per_rank, core_ids=list(range(NCORES)), trace=TRACE)
            break
        except Exception as e:  # transient runtime/collective hiccups
            last_exc = e
    if res is None:
        raise last_exc
    global LAST_RESULTS
    LAST_RESULTS = res
    outT_full = np.empty((C, N), dtype=np.float32)
    for r in range(NCORES):
        outT_full[r * CLOC + new2old[r]] = np.asarray(
            res.results[r]["outT"], dtype=np.float32)
    return np.ascontiguousarray(outT_full.T)
